# revision 3
# baseline (speedup 1.0000x reference)
"""GAT-style sparse attention layer on 8 TRN2 NeuronCores.

Row-shards N=8192 across 8 cores (1024 rows each). Each core:
  1. Wh_local = h_local @ W            (PE, bf16)
  2. AllGather Wh (bf16) + d-row (attn_dst scores, transposed locally)
  3. Per 128-row chunk x 2048-col block:
       Zm = 200*adj + D               (DVE scalar_tensor_tensor; D = d_j broadcast)
       L  = Lrelu(Zm + (s_i - 200))   (ACT, alpha=0.2; mask folds through:
                                       edges -> lrelu(z), non-edges -> 0.2z-40)
       P  = Exp(L) -> bf16            (ACT, accum_out = row sums for free)
       PT = PE-transpose(P)           (PSUM) -> SBUF
       acc += PT.T @ Wh_gathered      (PE, PSUM f32 accumulate)
     out = acc * (1/rowsum)
"""

import os
import sys

for _p in ("/opt/trn_rl_repo", "/opt/pypackages"):
    if _p not in sys.path and os.path.isdir(_p):
        sys.path.append(_p)

import ml_dtypes
import numpy as np

import concourse.bass as bass
import concourse.tile as tile
from concourse import bacc, mybir
from concourse.bass_utils import run_bass_kernel_spmd

F32 = mybir.dt.float32
BF16 = mybir.dt.bfloat16
AF = mybir.ActivationFunctionType
ALU = mybir.AluOpType

N = 8192
K_IN = 512
F_OUT = 256
P = 128
CORES = 8
L = N // CORES          # 1024 rows per core
NCH = L // P            # 8 row chunks per core
JB = 2048               # j-block width for elementwise phase
NJB = N // JB           # 4
MASK = 200.0            # additive mask offset (exp(z-200) underflows to 0)
ALPHA = 0.2             # leaky relu slope

_cache = {}


def _build():
    nc = bacc.Bacc(
        "TRN2",
        target_bir_lowering=False,
        debug=False,
        enable_asserts=False,
        num_devices=CORES,
    )

    hT_ext = nc.dram_tensor("hT", [K_IN, L], F32, kind="ExternalInput")
    adj_ext = nc.dram_tensor("adj", [L, N], BF16, kind="ExternalInput")
    w_ext = nc.dram_tensor("W", [K_IN, F_OUT], F32, kind="ExternalInput")
    asrc_ext = nc.dram_tensor("a_src", [F_OUT, 1], F32, kind="ExternalInput")
    adst_ext = nc.dram_tensor("a_dst", [F_OUT, 1], F32, kind="ExternalInput")
    out_ext = nc.dram_tensor("out", [L, F_OUT], F32, kind="ExternalOutput")

    ident_dram = nc.inline_tensor(
        np.eye(P, dtype=ml_dtypes.bfloat16), name="ident128"
    )

    KC = K_IN // P   # 4 k-chunks
    FC = F_OUT // P  # 2 f-chunks

    with tile.TileContext(nc) as tc:
        with (
            tc.tile_pool(name="keep", bufs=1) as keep,
            tc.tile_pool(name="dram", bufs=1, space="DRAM") as dram,
        ):
            # ---- long-lived tiles ----
            ident = keep.tile([P, P], BF16)
            nc.sync.dma_start(ident[:, :], ident_dram[:, :])
            D = keep.tile([P, N], BF16)          # d_j broadcast across partitions
            whg = keep.tile([P, (N // P) * F_OUT], BF16)  # gathered Wh, chunk jc at cols [jc*F : (jc+1)*F]
            s_m200 = keep.tile([P, NCH], F32)    # s_i - 200 bias, col per row-chunk

            wh_in = dram.tile([L, F_OUT], BF16)
            wh_all = dram.tile([N, F_OUT], BF16, addr_space="Shared")
            d_in = dram.tile([1, L], BF16)
            d_all = dram.tile([CORES, L], BF16, addr_space="Shared")

            # ---- phase A: Wh, WhT, s, d ----
            with (
                tc.tile_pool(name="setup", bufs=2) as sp,
                tc.tile_pool(name="setup_ps", bufs=1, space="PSUM") as spp,
                tc.tile_pool(name="whT_pool", bufs=1) as whp,
            ):
                hTb = []
                wb = []
                for kc in range(KC):
                    hT_f32 = sp.tile([P, L], F32, tag="hT_f32")
                    nc.sync.dma_start(hT_f32[:, :], hT_ext[kc * P:(kc + 1) * P, :])
                    t = whp.tile([P, L], BF16, name=f"hTb{kc}")
                    nc.vector.tensor_copy(t[:, :], hT_f32[:, :])
                    hTb.append(t)
                    w_f32 = sp.tile([P, F_OUT], F32, tag="w_f32")
                    nc.sync.dma_start(w_f32[:, :], w_ext[kc * P:(kc + 1) * P, :])
                    tw = whp.tile([P, F_OUT], BF16, name=f"wb{kc}")
                    nc.vector.tensor_copy(tw[:, :], w_f32[:, :])
                    wb.append(tw)

                avecs = []  # a_src chunks then a_dst chunks, bf16
                for name, ext in (("asrc", asrc_ext), ("adst", adst_ext)):
                    chunks = []
                    for fc in range(FC):
                        a_f32 = sp.tile([P, 1], F32, tag="a_f32")
                        nc.sync.dma_start(a_f32[:, :], ext[fc * P:(fc + 1) * P, :])
                        ab = whp.tile([P, 1], BF16, name=f"{name}{fc}")
                        nc.vector.tensor_copy(ab[:, :], a_f32[:, :])
                        chunks.append(ab)
                    avecs.append(chunks)
                asrcb, adstb = avecs

                # Wh_local chunks -> bounce DRAM (natural [i, f] layout)
                for c in range(NCH):
                    ps = spp.tile([P, F_OUT], F32, tag="wh_ps")
                    for kc in range(KC):
                        nc.tensor.matmul(
                            ps[:, :],
                            lhsT=hTb[kc][:, c * P:(c + 1) * P],
                            rhs=wb[kc][:, :],
                            start=(kc == 0),
                            stop=(kc == KC - 1),
                        )
                    whl = sp.tile([P, F_OUT], BF16, tag="whl")
                    nc.vector.tensor_copy(whl[:, :], ps[:, :])
                    nc.sync.dma_start(wh_in[c * P:(c + 1) * P, :], whl[:, :])

                # WhT chunks [f, i] (for s column / d row)
                whT = []
                for fc in range(FC):
                    ps = spp.tile([P, L], F32, tag="whT_ps")
                    for half in range(2):
                        hs = slice(half * 512, (half + 1) * 512)
                        for kc in range(KC):
                            nc.tensor.matmul(
                                ps[:, hs],
                                lhsT=wb[kc][:, fc * P:(fc + 1) * P],
                                rhs=hTb[kc][:, hs],
                                start=(kc == 0),
                                stop=(kc == KC - 1),
                            )
                    t = whp.tile([P, L], BF16, name=f"whT{fc}")
                    nc.vector.tensor_copy(t[:, :], ps[:, :])
                    whT.append(t)

                # s column [128, NCH]: s[i] = Wh[i, :] @ a_src
                scol_ps = spp.tile([P, NCH], F32, tag="scol_ps")
                for c in range(NCH):
                    for fc in range(FC):
                        nc.tensor.matmul(
                            scol_ps[:, c:c + 1],
                            lhsT=whT[fc][:, c * P:(c + 1) * P],
                            rhs=asrcb[fc][:, :],
                            start=(fc == 0),
                            stop=(fc == FC - 1),
                        )
                nc.vector.tensor_scalar_add(s_m200[:, :], scol_ps[:, :], -MASK)

                # d row [1, L]: d[j] = Wh[j, :] @ a_dst, as a row for broadcast
                drow_ps = spp.tile([1, L], F32, tag="drow_ps")
                for half in range(2):
                    hs = slice(half * 512, (half + 1) * 512)
                    for fc in range(FC):
                        nc.tensor.matmul(
                            drow_ps[:, hs],
                            lhsT=adstb[fc][:, :],
                            rhs=whT[fc][:, hs],
                            start=(fc == 0),
                            stop=(fc == FC - 1),
                        )
                drow_sb = sp.tile([1, L], BF16, tag="drow_sb")
                nc.vector.tensor_copy(drow_sb[:, :], drow_ps[:, :])
                nc.sync.dma_start(d_in[:, :], drow_sb[:, :])

            # ---- phase B: collectives ----
            rg = [list(range(CORES))]
            nc.gpsimd.collective_compute(
                "AllGather", ALU.bypass, replica_groups=rg,
                ins=[d_in.opt()], outs=[d_all.opt()],
            )
            nc.gpsimd.collective_compute(
                "AllGather", ALU.bypass, replica_groups=rg,
                ins=[wh_in.opt()], outs=[wh_all.opt()],
            )

            # D broadcast: D[:, jc*L:(jc+1)*L] = d_all[jc, :] on every partition
            for jc in range(CORES):
                nc.sync.dma_start(
                    D[:, jc * L:(jc + 1) * L],
                    d_all[jc:jc + 1, :].partition_broadcast(P),
                )
            # gathered Wh -> SBUF chunks
            for jc in range(N // P):
                nc.sync.dma_start(
                    whg[:, jc * F_OUT:(jc + 1) * F_OUT],
                    wh_all[jc * P:(jc + 1) * P, :],
                )

            # ---- phase C: main loop ----
            with (
                tc.tile_pool(name="adjp", bufs=3) as adjp,
                tc.tile_pool(name="zp", bufs=2) as zp,
                tc.tile_pool(name="lp", bufs=2) as lp,
                tc.tile_pool(name="pp", bufs=2) as pp,
                tc.tile_pool(name="ptsp", bufs=2) as ptsp,
                tc.tile_pool(name="smallp", bufs=2) as smallp,
                tc.tile_pool(name="accp", bufs=2, space="PSUM") as accp,
                tc.tile_pool(name="ttp", bufs=2, space="PSUM") as ttp,
            ):
                for c in range(NCH):
                    acc = accp.tile([P, F_OUT], F32, tag="acc")
                    rs_parts = smallp.tile([P, NJB], F32, tag="rs_parts")
                    for jb in range(NJB):
                        adj_t = adjp.tile([P, JB], BF16, tag="adj_t")
                        nc.sync.dma_start(
                            adj_t[:, :],
                            adj_ext[c * P:(c + 1) * P, jb * JB:(jb + 1) * JB],
                        )
                        zm = zp.tile([P, JB], F32, tag="zm")
                        nc.vector.scalar_tensor_tensor(
                            zm[:, :],
                            in0=adj_t[:, :],
                            scalar=MASK,
                            in1=D[:, jb * JB:(jb + 1) * JB],
                            op0=ALU.mult,
                            op1=ALU.add,
                        )
                        lr = lp.tile([P, JB], F32, tag="lr")
                        nc.scalar.activation(
                            lr[:, :], zm[:, :], AF.Prelu,
                            bias=s_m200[:, c:c + 1], scale=1.0, alpha=ALPHA,
                        )
                        pt = pp.tile([P, JB], BF16, tag="pt")
                        nc.scalar.activation(
                            pt[:, :], lr[:, :], AF.Exp,
                            accum_out=rs_parts[:, jb:jb + 1],
                        )
                        ptt = ttp.tile([P, JB], BF16, tag="ptt")
                        for k in range(JB // P):
                            nc.tensor.transpose(
                                ptt[:, k * P:(k + 1) * P],
                                pt[:, k * P:(k + 1) * P],
                                ident[:, :],
                            )
                        pts = ptsp.tile([P, JB], BF16, tag="pts")
                        nc.vector.tensor_copy(pts[:, :1024], ptt[:, :1024])
                        nc.vector.tensor_copy(pts[:, 1024:], ptt[:, 1024:])
                        for k in range(JB // P):
                            jc = jb * (JB // P) + k
                            nc.tensor.matmul(
                                acc[:, :],
                                lhsT=pts[:, k * P:(k + 1) * P],
                                rhs=whg[:, jc * F_OUT:(jc + 1) * F_OUT],
                                start=(jb == 0 and k == 0),
                                stop=(jb == NJB - 1 and k == JB // P - 1),
                            )
                    rs = smallp.tile([P, 1], F32, tag="rs")
                    nc.vector.tensor_reduce(
                        rs[:, :], rs_parts[:, :], axis=mybir.AxisListType.X, op=ALU.add
                    )
                    rsi = smallp.tile([P, 1], F32, tag="rsi")
                    nc.vector.reciprocal(rsi[:, :], rs[:, :])
                    outt = smallp.tile([P, F_OUT], F32, tag="outt")
                    nc.vector.tensor_scalar_mul(outt[:, :], acc[:, :], rsi[:, :])
                    nc.sync.dma_start(out_ext[c * P:(c + 1) * P, :], outt[:, :])

    nc.compile()
    return nc


def kernel(h, adj, W, a_src, a_dst):
    if "nc" not in _cache:
        _cache["nc"] = _build()
    nc = _cache["nc"]

    h = np.asarray(h, dtype=np.float32)
    W = np.asarray(W, dtype=np.float32)
    a_src = np.asarray(a_src, dtype=np.float32)
    a_dst = np.asarray(a_dst, dtype=np.float32)
    adj_b = np.asarray(adj != 0, dtype=ml_dtypes.bfloat16)

    in_maps = []
    for r in range(CORES):
        rows = slice(r * L, (r + 1) * L)
        in_maps.append({
            "hT": np.ascontiguousarray(h[rows].T),
            "adj": np.ascontiguousarray(adj_b[rows]),
            "W": W,
            "a_src": a_src,
            "a_dst": a_dst,
        })

    trace = bool(int(os.environ.get("KERNEL_TRACE", "0")))
    res = run_bass_kernel_spmd(
        nc, in_maps, core_ids=list(range(CORES)), trace=trace,
    )
    _cache["last_result"] = res
    out = np.concatenate([r["out"] for r in res.results], axis=0)
    return out


if __name__ == "__main__":
    rng = np.random.default_rng(0)
    h = rng.standard_normal((N, K_IN), dtype=np.float32)
    adj = (rng.random((N, N)) < 0.5).astype(np.int32)
    W = rng.standard_normal((K_IN, F_OUT), dtype=np.float32) * 0.05
    a_src = rng.standard_normal((F_OUT, 1), dtype=np.float32) * 0.09
    a_dst = rng.standard_normal((F_OUT, 1), dtype=np.float32) * 0.09
    out = kernel(h=h, adj=adj, W=W, a_src=a_src, a_dst=a_dst)
    print("out", out.shape, out.dtype, out[:2, :4])
